# revision 50
# baseline (speedup 1.0000x reference)
"""Trainium2 Bass kernel for multi-head attention (nn_Attention).

Problem: x[8, 32, 32, 768] -> MHA(12 heads, d=64) -> out[8, 32, 32, 768].

Sharding: pure data parallel. Batch B=8 maps 1:1 onto the 8 NeuronCores;
weights are replicated. No collectives.

Per-core algorithm (N=1024 tokens, C=768), all matmuls bf16 with fp32 PSUM
accumulation. v4 redesign around the v2/v3 trace findings: the PE runs its
queue strictly FIFO, so any matmul waiting on a PSUM-ring drain blocks all
later PE work. v2/v3 allocated scores psum from a shared 2-slot pool that
fillers also used, which collapsed each of the 48 steps into a serialized
exp -> scores -> exp ping-pong (~4.1us/step, 267us total). v4:

  1. PSUM layout (8 banks): scores ring psS = 3 x 1-bank slots holding
     [128,512] chunks (exp is issued per chunk, 4/step); fillers (qk/v/
     norm/bias/transpose lumps) get a dedicated 1-bank pool psF so they
     never perturb the scores ring; PV keeps 2 x 2-bank psum (psb).
  2. Step emission order: PV chunks (lag-5) first, fillers, scores LAST.
     scores(s) matmuls wait on exp(s-1) slot drains; placing them at the
     end of the step's PE stream means those drains completed while the
     PE ran pv/filler work -> no head-of-line stall, engines pipeline one
     step out of phase (ACT ~2.9us/step of exp, PE ~3.0us/step).
  3. Prologue: x streams in, f32->bf16 cast (DVE), bf16 PE-transposes with
     drains on the otherwise-idle ACT engine; only W tiles 6,0,1,7 staged
     before the loop; warmup matmuls split 8+8 with the second burst just
     before qk(6) so the PE clock gate (HAM) is open when the qk/scores
     stream begins. Remaining W/PW tiles + projections are loop filler.
  4. PV: out^T[d,i] + denominator row = [V|1].T @ E per 512-query chunk;
     reciprocal on DVE, 1/den broadcast by K=1 fp32r ones matmuls, fused
     into OTn in place; out = OTn.T @ PwT + proj_b per token tile.
"""

import os
import sys

for _p in ("/opt/trn_rl_repo",):
    if _p not in sys.path:
        sys.path.insert(0, _p)

import numpy as np

import concourse.bass as bass
from concourse import bacc
import concourse.mybir as mybir
from concourse.tile import TileContext

F32 = mybir.dt.float32
F32R = mybir.dt.float32r
BF16 = mybir.dt.bfloat16
EXP = mybir.ActivationFunctionType.Exp

P = 128
C = 768            # model dim
CT = C // P        # 6 c-tiles
N = 1024           # tokens per batch element
NT = N // P        # 8 token tiles
HEADS = 12
D = 64
OQK = 2 * C        # 1536 rows of q+k features
SCALE = D ** -0.5  # 0.125
HB = 512           # psum half-row chunk


def build_nc() -> bass.Bass:
    nc = bacc.Bacc(None, target_bir_lowering=False)
    x_d = nc.declare_dram_parameter("x", [N, C], F32, isOutput=False)
    qkvw_d = nc.declare_dram_parameter("qkv_w", [3 * C, C], F32, isOutput=False)
    qkvb_d = nc.declare_dram_parameter("qkv_b", [3 * C], F32, isOutput=False)
    projw_d = nc.declare_dram_parameter("proj_w", [C, C], F32, isOutput=False)
    projb_d = nc.declare_dram_parameter("proj_b", [C], F32, isOutput=False)
    out_d = nc.declare_dram_parameter("out", [N, C], F32, isOutput=True)

    with TileContext(nc) as tc:
        with (
            tc.tile_pool(name="const", bufs=1) as cpool,
            tc.tile_pool(name="ld", bufs=4) as ldp,
            tc.tile_pool(name="cv", bufs=6) as cvp,
            tc.tile_pool(name="xTp", bufs=1) as xtp,
            tc.tile_pool(name="wTp", bufs=1) as wtp,
            tc.tile_pool(name="pwp", bufs=1) as pwp,
            tc.tile_pool(name="qk", bufs=1) as qkp,
            tc.tile_pool(name="v", bufs=1) as vp,
            tc.tile_pool(name="ot", bufs=1) as otp,
            tc.tile_pool(name="e", bufs=10) as ep,
            tc.tile_pool(name="rec", bufs=2) as recp,
            tc.tile_pool(name="outs", bufs=2) as outp,
            tc.tile_pool(name="psS", bufs=3, space="PSUM") as psS,
            tc.tile_pool(name="psF", bufs=1, space="PSUM") as psF,
            tc.tile_pool(name="psb", bufs=2, space="PSUM") as psb,
        ):
            # ---------------- constants ----------------
            from concourse.masks import make_identity
            ident_f = cpool.tile([P, P], F32, tag="ident_f")
            make_identity(nc, ident_f)
            ident = cpool.tile([P, P], BF16, tag="ident")
            nc.vector.tensor_copy(ident, ident_f)
            zeros = cpool.tile([P, HB], BF16, tag="zeros")
            nc.gpsimd.memset(zeros, 0.0)

            def warmup(nmm):
                """Real (non-transpose) matmuls on zeros to open the PE
                HAM clock gate (transpose-mode ops do not count)."""
                ps_wu = psS.tile([P, HB], F32, tag="psS", name="ps_wu")
                for wi in range(nmm):
                    nc.tensor.matmul(
                        ps_wu, ident, zeros,
                        start=(wi == 0), stop=(wi == nmm - 1),
                    )
                nc.vector.tensor_copy(zeros, ps_wu)

            warmup(8)

            # ---------------- persistent activations ----------------
            xT = xtp.tile([P, CT, N], BF16, tag="xT")
            WT = wtp.tile([P, CT, 3 * C], BF16, tag="WT")
            PwT = pwp.tile([P, CT, C], BF16, tag="PwT")
            qkT = qkp.tile([P, HEADS, N], BF16, tag="qkT")
            V = vp.tile([P, NT, HEADS, D + 1], BF16, tag="V")
            OTn = otp.tile([P, CT, N], BF16, tag="OTn")
            nc.gpsimd.memset(V[:, :, :, D], 1.0)

            # ---------------- helpers ----------------
            def load_tile(dram_rows, cast="vector"):
                st = ldp.tile([P, C], F32, tag="ld")
                nc.sync.dma_start(st, dram_rows)
                bt = cvp.tile([P, C], BF16, tag="cv")
                if cast == "gpsimd":
                    nc.gpsimd.tensor_copy(bt, st)
                else:
                    nc.vector.tensor_copy(bt, st)
                return bt

            def trans_blocks(bt, dest_slice, pool, drain):
                """bf16 PE transpose of six 128x128 blocks into one psum
                bank, one batched copy out on `drain` (DVE in-loop, ACT in
                the prologue where it is otherwise idle)."""
                pst = pool.tile([P, CT * P], BF16, tag=pool.name, name="pst")
                for ct in range(CT):
                    nc.tensor.transpose(
                        pst[:, ct * P : (ct + 1) * P],
                        bt[:, ct * P : (ct + 1) * P],
                        ident,
                    )
                src = pst.rearrange("p (a b) -> p a b", b=P)
                if drain == "act":
                    nc.scalar.copy(dest_slice, src)
                else:
                    nc.vector.tensor_copy(dest_slice, src)

            def trans_blocks_mm(bt, dest_slice):
                """In-loop transpose via REGULAR matmul (block.T @ ident):
                numerically identical to transpose-mode, but counts as real
                PE activity for the HAM clock gate, so the transpose lumps
                no longer read as idle to the MID-window detector. Output
                is fp32 psum -> two [P,384] half-lumps, drained DVE/ACT."""
                for g in range(2):
                    pst = psF.tile([P, 3 * P], F32, tag="psF", name="pstmm")
                    for i in range(3):
                        ct = g * 3 + i
                        nc.tensor.matmul(
                            pst[:, i * P : (i + 1) * P],
                            bt[:, ct * P : (ct + 1) * P],
                            ident,
                            start=True,
                            stop=True,
                        )
                    dst = dest_slice[:, g * 3 : (g + 1) * 3, :]
                    srcg = pst.rearrange("p (a b) -> p a b", b=P)
                    if g == 0:
                        nc.vector.tensor_copy(dst, srcg)
                    else:
                        nc.scalar.copy(dst, srcg)

            staged = {}

            def load_x(nt):
                staged["x", nt] = load_tile(x_d[nt * P : (nt + 1) * P, :])

            def trans_x(nt):
                trans_blocks(
                    staged.pop(("x", nt)), xT[:, :, nt * P : (nt + 1) * P],
                    psS, "act",
                )

            def load_w(ot, cast="vector"):
                staged["w", ot] = load_tile(
                    qkvw_d[ot * P : (ot + 1) * P, :], cast=cast
                )

            def trans_w(ot, pool=None, drain="vector"):
                dest = WT[:, :, ot * P : (ot + 1) * P]
                if pool is None:
                    trans_blocks_mm(staged.pop(("w", ot)), dest)
                else:
                    trans_blocks(staged.pop(("w", ot)), dest, pool, drain)

            def load_pw(ct):
                staged["pw", ct] = load_tile(projw_d[ct * P : (ct + 1) * P, :])

            def trans_pw(ct):
                trans_blocks_mm(
                    staged.pop(("pw", ct)), PwT[:, :, ct * P : (ct + 1) * P]
                )

            def qk_tile(ot, pool=None, drain="vector", ics=(0, 1)):
                """Feature-major q/k projection for one 128-feature tile,
                one 512-token chunk per psum tile."""
                for ic in ics:
                    ps = (pool or psF).tile(
                        [P, HB], F32, tag=(pool or psF).name, name="ps_qk"
                    )
                    for ct in range(CT):
                        nc.tensor.matmul(
                            ps,
                            WT[:, ct, ot * P : (ot + 1) * P],
                            xT[:, ct, ic * HB : (ic + 1) * HB],
                            start=(ct == 0),
                            stop=(ct == CT - 1),
                        )
                    if drain == "act":
                        nc.scalar.activation(
                            qkT[:, ot, ic * HB : (ic + 1) * HB], ps,
                            mybir.ActivationFunctionType.Identity,
                            bias=bqk[:, ot : ot + 1],
                        )
                    else:
                        nc.vector.tensor_scalar_add(
                            qkT[:, ot, ic * HB : (ic + 1) * HB], ps,
                            bqk[:, ot : ot + 1],
                        )

            def v_proj(nt):
                """Token-major V projection for one token tile, split into
                512- and 256-wide output chunks (heads 0-7 / 8-11)."""
                for o0, ow, h0, h1 in ((0, 512, 0, 8), (512, 256, 8, 12)):
                    ps = psF.tile([P, ow], F32, tag="psF", name="ps_v")
                    for ct in range(CT):
                        nc.tensor.matmul(
                            ps,
                            xT[:, ct, nt * P : (nt + 1) * P],
                            WT[:, ct, OQK + o0 : OQK + o0 + ow],
                            start=(ct == 0),
                            stop=(ct == CT - 1),
                        )
                    nc.vector.tensor_add(
                        V[:, nt, h0:h1, 0:D],
                        ps.rearrange("p (h d) -> p h d", d=D),
                        bv_bc[:, h0:h1, :],
                    )

            Et = {}  # (pair, half, jt//2) -> E tile [P, 2, N]

            def scores_chunk(pair, jt):
                """S^T and exp for both heads of a pair, one key tile.
                Four [128,512] psum chunks from the 3-slot psS ring; exp
                issued per chunk so each slot frees after ~720ns of ACT."""
                chunks = []
                for ic in range(2):
                    for half in (0, 1):
                        lo = half * D
                        ps = psS.tile(
                            [P, HB], F32, tag="psS", name=f"ps_s{half}_{ic}"
                        )
                        nc.tensor.matmul(
                            ps,
                            qkT[lo : lo + D, CT + pair, jt * P : (jt + 1) * P],
                            qkT[lo : lo + D, pair, ic * HB : (ic + 1) * HB],
                            start=True,
                            stop=True,
                            tile_position=(lo, 0),
                        )
                        chunks.append((half, ic, ps))
                for half, ic, ps in chunks:
                    nc.scalar.activation(
                        Et[(pair, half, jt // 2)][
                            :, jt % 2, ic * HB : (ic + 1) * HB
                        ],
                        ps, EXP, scale=SCALE,
                    )

            def pv_chunk(h, jt, pspv):
                for ic in range(2):
                    nc.tensor.matmul(
                        pspv[0 : D + 1, ic * HB : (ic + 1) * HB],
                        V[:, jt, h, :],
                        Et[(h // 2, h % 2, jt // 2)][
                            :, jt % 2, ic * HB : (ic + 1) * HB
                        ],
                        start=(jt == 0),
                        stop=(jt == NT - 1),
                    )

            def pv_finish_psum(h, pspv, den):
                """PSUM reads only (den row + numerator) so the psb slot
                frees as soon as possible - the next pair's PV matmuls wait
                on it and, PE being FIFO, would block everything behind."""
                nc.vector.tensor_copy(den, pspv[D : D + 1, :])
                nc.vector.tensor_copy(
                    OTn[(h % 2) * D : (h % 2) * D + D, h // 2, :], pspv[0:D, :]
                )

            def pv_finish_rec(h, den, rec, rec_r):
                """SBUF-only recip chain; can lag the psum drain."""
                nc.vector.reciprocal_approx_fast(rec, den)
                nc.vector.tensor_copy(rec_r, rec)

            def norm_pair(pair, recA, recB):
                """Broadcast 1/den of both heads (pair-packed fp32r K=1
                matmuls) and scale the numerators in place."""
                for ic in range(2):
                    s = slice(ic * HB, (ic + 1) * HB)
                    psbc = psF.tile([P, HB], F32, tag="psF", name="ps_bc")
                    nc.tensor.matmul(
                        psbc, onesA, recA[:, s], start=True, stop=False,
                    )
                    nc.tensor.matmul(
                        psbc, onesB, recB[:, s], start=False, stop=True,
                    )
                    nc.vector.tensor_mul(
                        OTn[:, pair, s], OTn[:, pair, s], psbc
                    )

            def proj_tile(it):
                """Proj for one token tile (epilogue; psS ring is free)."""
                outt = outp.tile([P, C], F32, tag="out")
                for o0, ow in ((0, 512), (512, 256)):
                    ps = psS.tile([P, ow], F32, tag="psS", name="ps_o")
                    for ct in range(CT):
                        nc.tensor.matmul(
                            ps,
                            OTn[:, ct, it * P : (it + 1) * P],
                            PwT[:, ct, o0 : o0 + ow],
                            start=(ct == 0),
                            stop=(ct == CT - 1),
                        )
                    nc.vector.tensor_add(
                        outt[:, o0 : o0 + ow], ps, pb_bc[:, o0 : o0 + ow]
                    )
                nc.sync.dma_start(out_d[it * P : (it + 1) * P, :], outt)

            # ---------------- emission: prologue ----------------
            ones_st = cpool.tile([1, P], F32, tag="ones_st")
            nc.gpsimd.memset(ones_st, 1.0)
            ones_r = cpool.tile([1, P], F32R, tag="ones_r")
            nc.vector.tensor_copy(ones_r, ones_st)
            onesA_st = cpool.tile([1, P], F32, tag="onesA_st")
            nc.gpsimd.memset(onesA_st, 0.0)
            nc.gpsimd.memset(onesA_st[0:1, 0:D], 1.0)
            onesB_st = cpool.tile([1, P], F32, tag="onesB_st")
            nc.gpsimd.memset(onesB_st, 0.0)
            nc.gpsimd.memset(onesB_st[0:1, D:P], 1.0)
            onesA = cpool.tile([1, P], F32R, tag="onesA")
            nc.vector.tensor_copy(onesA, onesA_st)
            onesB = cpool.tile([1, P], F32R, tag="onesB")
            nc.vector.tensor_copy(onesB, onesB_st)

            load_x(0)
            load_x(1)
            load_x(2)
            for nt in range(3, NT):
                load_x(nt)
                trans_x(nt - 3)

            # bias DMAs after the x stream (small/strided; keep off Q1 head)
            bqk = cpool.tile([P, HEADS], F32, tag="bqk")
            nc.sync.dma_start(bqk, qkvb_d[0:OQK].rearrange("(t p) -> p t", p=P))
            bv_st = cpool.tile([1, C], F32, tag="bv_st")
            nc.sync.dma_start(bv_st, qkvb_d[None, OQK : 3 * C])
            pb_st = cpool.tile([1, C], F32, tag="pb_st")
            nc.sync.dma_start(pb_st, projb_d[None, :])
            bv_r = cpool.tile([1, C], F32R, tag="bv_r")
            nc.vector.tensor_copy(bv_r, bv_st)
            pb_r = cpool.tile([1, C], F32R, tag="pb_r")
            nc.vector.tensor_copy(pb_r, pb_st)
            bv3 = cpool.tile([P, HEADS, D], BF16, tag="bv3")
            bv_bc = bv3  # [P, h, d] view for v_proj drains
            pb_bc = cpool.tile([P, C], BF16, tag="pb_bc")

            load_w(6)
            trans_x(NT - 3)
            load_w(0)
            trans_x(NT - 2)
            load_w(1)
            trans_x(NT - 1)
            load_w(7)
            trans_w(6, pool=psS, drain="act")
            trans_w(0, pool=psS, drain="act")
            trans_w(1, pool=psS, drain="act")
            trans_w(7, pool=psS, drain="act")
            warmup(8)
            qk_tile(6, pool=psS, drain="act")
            qk_tile(0, pool=psS, drain="act")

            def bias_bcast(src, dst2d):
                """Broadcast a [1,C] bias row to [P,C] (K=1 fp32r matmul)."""
                for o0, ow in ((0, 512), (512, 256)):
                    psx = psF.tile([P, ow], F32, tag="psF", name="ps_bias")
                    nc.tensor.matmul(
                        psx, ones_r, src[:, o0 : o0 + ow],
                        start=True, stop=True,
                    )
                    nc.vector.tensor_copy(dst2d[:, o0 : o0 + ow], psx)

            pspv = {}          # head -> psum tile
            recs = {}          # head -> reciprocal row (f32r)

            LAG = 5

            def do_pv_step(s):
                q, jtp = (s - LAG) // 8, (s - LAG) % 8
                if jtp == 0:
                    pspv[2 * q] = psb.tile([P, N], F32, tag="psb", name="ps_pv")
                    pspv[2 * q + 1] = psb.tile([P, N], F32, tag="psb", name="ps_pv")
                heads = (2 * q, 2 * q + 1)
                if q == 5 and jtp >= NT - 2:
                    heads = (2 * q + 1, 2 * q)
                for h in heads:
                    pv_chunk(h, jtp, pspv[h])
                if jtp == NT - 1:
                    dens = {}
                    for h in heads:
                        dens[h] = recp.tile([1, N], F32, tag="den", name=f"den_{h}")
                        pv_finish_psum(h, pspv[h], dens[h])
                    for h in heads:
                        rec = recp.tile([1, N], F32, tag="rec", name=f"rec_{h}")
                        recs[h] = recp.tile(
                            [1, N], F32R, tag="rec_r", name=f"rec_r_{h}"
                        )
                        pv_finish_rec(h, dens[h], rec, recs[h])

            # ---------------- 48-step loop ----------------
            # Filler schedule (step -> work). Deps: v-block transposes and
            # v_proj land in the first 12 steps (pv of pair 0 needs V tile
            # jtp by step 5+jtp), qk tiles for pair p+1 inside pair p, pw
            # staging spread over the light tail.
            LOADW = {
                1: (12, 13), 2: (14, 15), 3: (16, 17), 5: (2,), 6: (8,),
                9: (3,), 10: (9,), 13: (4,), 15: (10,), 17: (5,), 18: (11,),
            }
            LOADPW = {26: (0,), 28: (1,), 30: (2,), 32: (3,), 34: (4,), 36: (5,)}
            TRANSW = {
                2: (12, 13), 3: (14, 15), 4: (16, 17), 7: (2,), 8: (8,),
                11: (3,), 13: (9,), 16: (4,), 17: (10,), 19: (5,), 20: (11,),
            }
            TRANSPW = {33: (0,), 34: (1,), 36: (2,), 37: (3,), 40: (4,), 41: (5,)}
            QKS = {0: 7, 1: 1, 12: 2, 14: 8, 18: 3, 21: 9, 26: 4, 29: 10,
                   35: 5, 38: 11}
            VS = {s: s - 4 for s in range(4, 12)}

            for s in range(48):
                pair, jt = s // 8, s % 8
                if jt % 2 == 0:
                    for half in (0, 1):
                        Et[(pair, half, jt // 2)] = ep.tile(
                            [P, 2, N], BF16, tag="E", name=f"E_{pair}_{half}_{jt // 2}"
                        )
                # PV first (deps met 5 steps ago), fillers, scores LAST so
                # the PE never head-of-line blocks on an exp slot drain.
                if s >= LAG:
                    do_pv_step(s)
                for ot in TRANSW.get(s, ()):
                    trans_w(ot)
                for ct in TRANSPW.get(s, ()):
                    trans_pw(ct)
                if s == 3:
                    bias_bcast(bv_r, bv3.rearrange("p h d -> p (h d)"))
                if s in VS:
                    v_proj(VS[s])
                if s in QKS:
                    qk_tile(QKS[s])
                if s == 19:
                    bias_bcast(pb_r, pb_bc)
                for ot in LOADW.get(s, ()):
                    load_w(ot)
                for ct in LOADPW.get(s, ()):
                    load_pw(ct)
                # normalize previous pair
                if pair >= 1 and jt == 7:
                    norm_pair(pair - 1, recs[2 * (pair - 1)], recs[2 * (pair - 1) + 1])
                scores_chunk(pair, jt)

            for s in range(48, 48 + LAG):
                do_pv_step(s)
            norm_pair(5, recs[10], recs[11])
            for it in range(NT):
                proj_tile(it)

    nc.compile()
    return nc


_NC_CACHE = None


def _get_nc():
    global _NC_CACHE
    if _NC_CACHE is None:
        _NC_CACHE = build_nc()
    return _NC_CACHE


def run(inputs, trace=False, tmpdir=None):
    """Run on 8 NeuronCores; returns (out[8,32,32,768], BassKernelResults)."""
    from concourse.bass_utils import run_bass_kernel_spmd

    x = np.asarray(inputs["x"], dtype=np.float32)
    B, H, W, Cc = x.shape
    xf = np.ascontiguousarray(x.reshape(B, H * W, Cc))
    qkv_w = np.ascontiguousarray(np.asarray(inputs["qkv_w"], dtype=np.float32))
    qkv_b = np.ascontiguousarray(np.asarray(inputs["qkv_b"], dtype=np.float32))
    proj_w = np.ascontiguousarray(np.asarray(inputs["proj_w"], dtype=np.float32))
    proj_b = np.ascontiguousarray(np.asarray(inputs["proj_b"], dtype=np.float32))

    nc = _get_nc()
    in_maps = [
        {
            "x": xf[b],
            "qkv_w": qkv_w,
            "qkv_b": qkv_b,
            "proj_w": proj_w,
            "proj_b": proj_b,
        }
        for b in range(B)
    ]
    res = run_bass_kernel_spmd(nc, in_maps, list(range(B)), trace=trace, tmpdir=tmpdir)
    out = np.stack([res.results[b]["out"] for b in range(B)])
    return out.reshape(B, H, W, Cc).astype(np.float32), res


def kernel(x, qkv_w, qkv_b, proj_w, proj_b):
    out, _ = run(
        {
            "x": x,
            "qkv_w": qkv_w,
            "qkv_b": qkv_b,
            "proj_w": proj_w,
            "proj_b": proj_b,
        }
    )
    return out


# revision 52
# speedup vs baseline: 1.1033x; 1.1033x over previous
"""Trainium2 Bass kernel for multi-head attention (nn_Attention).

Problem: x[8, 32, 32, 768] -> MHA(12 heads, d=64) -> out[8, 32, 32, 768].

Sharding: pure data parallel. Batch B=8 maps 1:1 onto the 8 NeuronCores;
weights are replicated. No collectives.

Per-core algorithm (N=1024 tokens, C=768), all matmuls bf16 with fp32 PSUM
accumulation. v4 redesign around the v2/v3 trace findings: the PE runs its
queue strictly FIFO, so any matmul waiting on a PSUM-ring drain blocks all
later PE work. v2/v3 allocated scores psum from a shared 2-slot pool that
fillers also used, which collapsed each of the 48 steps into a serialized
exp -> scores -> exp ping-pong (~4.1us/step, 267us total). v4:

  1. PSUM layout (8 banks): scores ring psS = 3 x 1-bank slots holding
     [128,512] chunks (exp is issued per chunk, 4/step); fillers (qk/v/
     norm/bias/transpose lumps) get a dedicated 1-bank pool psF so they
     never perturb the scores ring; PV keeps 2 x 2-bank psum (psb).
  2. Step emission order: PV chunks (lag-5) first, fillers, scores LAST.
     scores(s) matmuls wait on exp(s-1) slot drains; placing them at the
     end of the step's PE stream means those drains completed while the
     PE ran pv/filler work -> no head-of-line stall, engines pipeline one
     step out of phase (ACT ~2.9us/step of exp, PE ~3.0us/step).
  3. Prologue: x streams in, f32->bf16 cast (DVE), bf16 PE-transposes with
     drains on the otherwise-idle ACT engine; only W tiles 6,0,1,7 staged
     before the loop; warmup matmuls split 8+8 with the second burst just
     before qk(6) so the PE clock gate (HAM) is open when the qk/scores
     stream begins. Remaining W/PW tiles + projections are loop filler.
  4. PV: out^T[d,i] + denominator row = [V|1].T @ E per 512-query chunk;
     reciprocal on DVE, 1/den broadcast by K=1 fp32r ones matmuls, fused
     into OTn in place; out = OTn.T @ PwT + proj_b per token tile.
"""

import os
import sys

for _p in ("/opt/trn_rl_repo",):
    if _p not in sys.path:
        sys.path.insert(0, _p)

import numpy as np

import concourse.bass as bass
from concourse import bacc
import concourse.mybir as mybir
from concourse.tile import TileContext

F32 = mybir.dt.float32
F32R = mybir.dt.float32r
BF16 = mybir.dt.bfloat16
EXP = mybir.ActivationFunctionType.Exp

P = 128
C = 768            # model dim
CT = C // P        # 6 c-tiles
N = 1024           # tokens per batch element
NT = N // P        # 8 token tiles
HEADS = 12
D = 64
OQK = 2 * C        # 1536 rows of q+k features
SCALE = D ** -0.5  # 0.125
HB = 512           # psum half-row chunk


def build_nc() -> bass.Bass:
    nc = bacc.Bacc(None, target_bir_lowering=False)
    x_d = nc.declare_dram_parameter("x", [N, C], F32, isOutput=False)
    qkvw_d = nc.declare_dram_parameter("qkv_w", [3 * C, C], F32, isOutput=False)
    qkvb_d = nc.declare_dram_parameter("qkv_b", [3 * C], F32, isOutput=False)
    projw_d = nc.declare_dram_parameter("proj_w", [C, C], F32, isOutput=False)
    projb_d = nc.declare_dram_parameter("proj_b", [C], F32, isOutput=False)
    out_d = nc.declare_dram_parameter("out", [N, C], F32, isOutput=True)

    with TileContext(nc) as tc:
        with (
            tc.tile_pool(name="const", bufs=1) as cpool,
            tc.tile_pool(name="ld", bufs=4) as ldp,
            tc.tile_pool(name="cv", bufs=6) as cvp,
            tc.tile_pool(name="xTp", bufs=1) as xtp,
            tc.tile_pool(name="wTp", bufs=1) as wtp,
            tc.tile_pool(name="pwp", bufs=1) as pwp,
            tc.tile_pool(name="qk", bufs=1) as qkp,
            tc.tile_pool(name="v", bufs=1) as vp,
            tc.tile_pool(name="ot", bufs=1) as otp,
            tc.tile_pool(name="e", bufs=10) as ep,
            tc.tile_pool(name="rec", bufs=2) as recp,
            tc.tile_pool(name="outs", bufs=2) as outp,
            tc.tile_pool(name="psS", bufs=3, space="PSUM") as psS,
            tc.tile_pool(name="psF", bufs=1, space="PSUM") as psF,
            tc.tile_pool(name="psb", bufs=2, space="PSUM") as psb,
        ):
            # ---------------- constants ----------------
            from concourse.masks import make_identity
            ident_f = cpool.tile([P, P], F32, tag="ident_f")
            make_identity(nc, ident_f)
            ident = cpool.tile([P, P], BF16, tag="ident")
            nc.vector.tensor_copy(ident, ident_f)
            zeros = cpool.tile([P, HB], BF16, tag="zeros")
            nc.gpsimd.memset(zeros, 0.0)

            def warmup(nmm):
                """Real (non-transpose) matmuls on zeros to open the PE
                HAM clock gate (transpose-mode ops do not count)."""
                ps_wu = psS.tile([P, HB], F32, tag="psS", name="ps_wu")
                for wi in range(nmm):
                    nc.tensor.matmul(
                        ps_wu, ident, zeros,
                        start=(wi == 0), stop=(wi == nmm - 1),
                    )
                nc.vector.tensor_copy(zeros, ps_wu)

            warmup(8)

            # ---------------- persistent activations ----------------
            xT = xtp.tile([P, CT, N], BF16, tag="xT")
            WT = wtp.tile([P, CT, 3 * C], BF16, tag="WT")
            PwT = pwp.tile([P, CT, C], BF16, tag="PwT")
            qkT = qkp.tile([P, HEADS, N], BF16, tag="qkT")
            V = vp.tile([P, NT, HEADS, D + 1], BF16, tag="V")
            OTn = otp.tile([P, CT, N], BF16, tag="OTn")
            nc.gpsimd.memset(V[:, :, :, D], 1.0)

            # ---------------- helpers ----------------
            def load_tile(dram_rows, cast="vector"):
                st = ldp.tile([P, C], F32, tag="ld")
                nc.sync.dma_start(st, dram_rows)
                bt = cvp.tile([P, C], BF16, tag="cv")
                if cast == "gpsimd":
                    nc.gpsimd.tensor_copy(bt, st)
                else:
                    nc.vector.tensor_copy(bt, st)
                return bt

            def trans_blocks(bt, dest_slice, pool, drain):
                """bf16 PE transpose of six 128x128 blocks into one psum
                bank, one batched copy out on `drain` (DVE in-loop, ACT in
                the prologue where it is otherwise idle)."""
                pst = pool.tile([P, CT * P], BF16, tag=pool.name, name="pst")
                for ct in range(CT):
                    nc.tensor.transpose(
                        pst[:, ct * P : (ct + 1) * P],
                        bt[:, ct * P : (ct + 1) * P],
                        ident,
                    )
                src = pst.rearrange("p (a b) -> p a b", b=P)
                if drain == "act":
                    nc.scalar.copy(dest_slice, src)
                else:
                    nc.vector.tensor_copy(dest_slice, src)

            staged = {}

            def load_x(nt):
                staged["x", nt] = load_tile(x_d[nt * P : (nt + 1) * P, :])

            def trans_x(nt):
                trans_blocks(
                    staged.pop(("x", nt)), xT[:, :, nt * P : (nt + 1) * P],
                    psS, "act",
                )

            def load_w(ot, cast="vector"):
                staged["w", ot] = load_tile(
                    qkvw_d[ot * P : (ot + 1) * P, :], cast=cast
                )

            def trans_w(ot, pool=None, drain="vector"):
                trans_blocks(
                    staged.pop(("w", ot)), WT[:, :, ot * P : (ot + 1) * P],
                    pool or psF, drain,
                )

            def load_pw(ct):
                staged["pw", ct] = load_tile(projw_d[ct * P : (ct + 1) * P, :])

            def trans_pw(ct):
                trans_blocks(
                    staged.pop(("pw", ct)), PwT[:, :, ct * P : (ct + 1) * P],
                    psF, "vector",
                )

            def qk_tile(ot, pool=None, drain="vector", ics=(0, 1)):
                """Feature-major q/k projection for one 128-feature tile,
                one 512-token chunk per psum tile."""
                for ic in ics:
                    ps = (pool or psF).tile(
                        [P, HB], F32, tag=(pool or psF).name, name="ps_qk"
                    )
                    for ct in range(CT):
                        nc.tensor.matmul(
                            ps,
                            WT[:, ct, ot * P : (ot + 1) * P],
                            xT[:, ct, ic * HB : (ic + 1) * HB],
                            start=(ct == 0),
                            stop=(ct == CT - 1),
                        )
                    if drain == "act":
                        nc.scalar.activation(
                            qkT[:, ot, ic * HB : (ic + 1) * HB], ps,
                            mybir.ActivationFunctionType.Identity,
                            bias=bqk[:, ot : ot + 1],
                        )
                    else:
                        nc.vector.tensor_scalar_add(
                            qkT[:, ot, ic * HB : (ic + 1) * HB], ps,
                            bqk[:, ot : ot + 1],
                        )

            def v_proj(nt):
                """Token-major V projection for one token tile, split into
                512- and 256-wide output chunks (heads 0-7 / 8-11)."""
                for o0, ow, h0, h1 in ((0, 512, 0, 8), (512, 256, 8, 12)):
                    ps = psF.tile([P, ow], F32, tag="psF", name="ps_v")
                    for ct in range(CT):
                        nc.tensor.matmul(
                            ps,
                            xT[:, ct, nt * P : (nt + 1) * P],
                            WT[:, ct, OQK + o0 : OQK + o0 + ow],
                            start=(ct == 0),
                            stop=(ct == CT - 1),
                        )
                    nc.vector.tensor_add(
                        V[:, nt, h0:h1, 0:D],
                        ps.rearrange("p (h d) -> p h d", d=D),
                        bv_bc[:, h0:h1, :],
                    )

            Et = {}  # (pair, half, jt//2) -> E tile [P, 2, N]

            def scores_chunk(pair, jt):
                """S^T and exp for both heads of a pair, one key tile.
                Four [128,512] psum chunks from the 3-slot psS ring; exp
                issued per chunk so each slot frees after ~720ns of ACT."""
                chunks = []
                for ic in range(2):
                    for half in (0, 1):
                        lo = half * D
                        ps = psS.tile(
                            [P, HB], F32, tag="psS", name=f"ps_s{half}_{ic}"
                        )
                        nc.tensor.matmul(
                            ps,
                            qkT[lo : lo + D, CT + pair, jt * P : (jt + 1) * P],
                            qkT[lo : lo + D, pair, ic * HB : (ic + 1) * HB],
                            start=True,
                            stop=True,
                            tile_position=(lo, 0),
                        )
                        chunks.append((half, ic, ps))
                for half, ic, ps in chunks:
                    nc.scalar.activation(
                        Et[(pair, half, jt // 2)][
                            :, jt % 2, ic * HB : (ic + 1) * HB
                        ],
                        ps, EXP, scale=SCALE,
                    )

            def pv_chunk(h, jt, pspv):
                for ic in range(2):
                    nc.tensor.matmul(
                        pspv[0 : D + 1, ic * HB : (ic + 1) * HB],
                        V[:, jt, h, :],
                        Et[(h // 2, h % 2, jt // 2)][
                            :, jt % 2, ic * HB : (ic + 1) * HB
                        ],
                        start=(jt == 0),
                        stop=(jt == NT - 1),
                    )

            def pv_finish_psum(h, pspv, den):
                """PSUM reads only (den row + numerator) so the psb slot
                frees as soon as possible - the next pair's PV matmuls wait
                on it and, PE being FIFO, would block everything behind."""
                nc.vector.tensor_copy(den, pspv[D : D + 1, :])
                nc.vector.tensor_copy(
                    OTn[(h % 2) * D : (h % 2) * D + D, h // 2, :], pspv[0:D, :]
                )

            def pv_finish_rec(h, den, rec, rec_r):
                """SBUF-only recip chain; can lag the psum drain."""
                nc.vector.reciprocal_approx_fast(rec, den)
                nc.vector.tensor_copy(rec_r, rec)

            def norm_pair(pair, recA, recB):
                """Broadcast 1/den of both heads (pair-packed fp32r K=1
                matmuls) and scale the numerators in place."""
                for ic in range(2):
                    s = slice(ic * HB, (ic + 1) * HB)
                    psbc = psF.tile([P, HB], F32, tag="psF", name="ps_bc")
                    nc.tensor.matmul(
                        psbc, onesA, recA[:, s], start=True, stop=False,
                    )
                    nc.tensor.matmul(
                        psbc, onesB, recB[:, s], start=False, stop=True,
                    )
                    nc.vector.tensor_mul(
                        OTn[:, pair, s], OTn[:, pair, s], psbc
                    )

            def proj_tile(it):
                """Proj for one token tile (epilogue; psS ring is free)."""
                outt = outp.tile([P, C], F32, tag="out")
                for o0, ow in ((0, 512), (512, 256)):
                    ps = psS.tile([P, ow], F32, tag="psS", name="ps_o")
                    for ct in range(CT):
                        nc.tensor.matmul(
                            ps,
                            OTn[:, ct, it * P : (it + 1) * P],
                            PwT[:, ct, o0 : o0 + ow],
                            start=(ct == 0),
                            stop=(ct == CT - 1),
                        )
                    nc.vector.tensor_add(
                        outt[:, o0 : o0 + ow], ps, pb_bc[:, o0 : o0 + ow]
                    )
                nc.sync.dma_start(out_d[it * P : (it + 1) * P, :], outt)

            # ---------------- emission: prologue ----------------
            ones_st = cpool.tile([1, P], F32, tag="ones_st")
            nc.gpsimd.memset(ones_st, 1.0)
            ones_r = cpool.tile([1, P], F32R, tag="ones_r")
            nc.vector.tensor_copy(ones_r, ones_st)
            onesA_st = cpool.tile([1, P], F32, tag="onesA_st")
            nc.gpsimd.memset(onesA_st, 0.0)
            nc.gpsimd.memset(onesA_st[0:1, 0:D], 1.0)
            onesB_st = cpool.tile([1, P], F32, tag="onesB_st")
            nc.gpsimd.memset(onesB_st, 0.0)
            nc.gpsimd.memset(onesB_st[0:1, D:P], 1.0)
            onesA = cpool.tile([1, P], F32R, tag="onesA")
            nc.vector.tensor_copy(onesA, onesA_st)
            onesB = cpool.tile([1, P], F32R, tag="onesB")
            nc.vector.tensor_copy(onesB, onesB_st)

            # W tiles 6,0,1,7 FIRST: they arrive ~10us in and transpose
            # under the x stream, so qk fires as soon as the last x tile
            # lands instead of serializing W staging after it
            load_w(6)
            load_w(0)
            load_w(1)
            load_w(7)
            load_x(0)
            load_x(1)
            trans_w(6, pool=psS, drain="act")
            load_x(2)
            trans_w(0, pool=psS, drain="act")
            load_x(3)
            trans_w(1, pool=psS, drain="act")
            load_x(4)
            trans_w(7, pool=psS, drain="act")
            load_x(5)
            trans_x(0)
            load_x(6)
            trans_x(1)
            load_x(7)
            trans_x(2)

            # bias DMAs after the big loads (small/strided)
            bqk = cpool.tile([P, HEADS], F32, tag="bqk")
            nc.sync.dma_start(bqk, qkvb_d[0:OQK].rearrange("(t p) -> p t", p=P))
            bv_st = cpool.tile([1, C], F32, tag="bv_st")
            nc.sync.dma_start(bv_st, qkvb_d[None, OQK : 3 * C])
            pb_st = cpool.tile([1, C], F32, tag="pb_st")
            nc.sync.dma_start(pb_st, projb_d[None, :])
            bv_r = cpool.tile([1, C], F32R, tag="bv_r")
            nc.vector.tensor_copy(bv_r, bv_st)
            pb_r = cpool.tile([1, C], F32R, tag="pb_r")
            nc.vector.tensor_copy(pb_r, pb_st)
            bv3 = cpool.tile([P, HEADS, D], BF16, tag="bv3")
            bv_bc = bv3  # [P, h, d] view for v_proj drains
            pb_bc = cpool.tile([P, C], BF16, tag="pb_bc")

            trans_x(3)
            trans_x(4)
            trans_x(5)
            trans_x(6)
            trans_x(7)
            warmup(8)
            qk_tile(6, pool=psS, drain="act")
            qk_tile(0, pool=psS, drain="act")

            def bias_bcast(src, dst2d):
                """Broadcast a [1,C] bias row to [P,C] (K=1 fp32r matmul)."""
                for o0, ow in ((0, 512), (512, 256)):
                    psx = psF.tile([P, ow], F32, tag="psF", name="ps_bias")
                    nc.tensor.matmul(
                        psx, ones_r, src[:, o0 : o0 + ow],
                        start=True, stop=True,
                    )
                    nc.vector.tensor_copy(dst2d[:, o0 : o0 + ow], psx)

            pspv = {}          # head -> psum tile
            recs = {}          # head -> reciprocal row (f32r)

            LAG = 5

            def do_pv_step(s):
                q, jtp = (s - LAG) // 8, (s - LAG) % 8
                if jtp == 0:
                    pspv[2 * q] = psb.tile([P, N], F32, tag="psb", name="ps_pv")
                    pspv[2 * q + 1] = psb.tile([P, N], F32, tag="psb", name="ps_pv")
                heads = (2 * q, 2 * q + 1)
                if q == 5 and jtp >= NT - 2:
                    heads = (2 * q + 1, 2 * q)
                for h in heads:
                    pv_chunk(h, jtp, pspv[h])
                if jtp == NT - 1:
                    dens = {}
                    for h in heads:
                        dens[h] = recp.tile([1, N], F32, tag="den", name=f"den_{h}")
                        pv_finish_psum(h, pspv[h], dens[h])
                    for h in heads:
                        rec = recp.tile([1, N], F32, tag="rec", name=f"rec_{h}")
                        recs[h] = recp.tile(
                            [1, N], F32R, tag="rec_r", name=f"rec_r_{h}"
                        )
                        pv_finish_rec(h, dens[h], rec, recs[h])

            # ---------------- 48-step loop ----------------
            # Filler schedule (step -> work). Deps: v-block transposes and
            # v_proj land in the first 12 steps (pv of pair 0 needs V tile
            # jtp by step 5+jtp), qk tiles for pair p+1 inside pair p, pw
            # staging spread over the light tail.
            LOADW = {
                1: (12, 13), 2: (14, 15), 3: (16, 17), 5: (2,), 6: (8,),
                9: (3,), 10: (9,), 13: (4,), 15: (10,), 17: (5,), 18: (11,),
            }
            LOADPW = {26: (0,), 28: (1,), 30: (2,), 32: (3,), 34: (4,), 36: (5,)}
            TRANSW = {
                2: (12, 13), 3: (14, 15), 4: (16, 17), 7: (2,), 8: (8,),
                11: (3,), 13: (9,), 16: (4,), 17: (10,), 19: (5,), 20: (11,),
            }
            TRANSPW = {33: (0,), 34: (1,), 36: (2,), 37: (3,), 40: (4,), 41: (5,)}
            QKS = {0: 7, 1: 1, 12: 2, 14: 8, 18: 3, 21: 9, 26: 4, 29: 10,
                   35: 5, 38: 11}
            VS = {s: s - 4 for s in range(4, 12)}

            for s in range(48):
                pair, jt = s // 8, s % 8
                if jt % 2 == 0:
                    for half in (0, 1):
                        Et[(pair, half, jt // 2)] = ep.tile(
                            [P, 2, N], BF16, tag="E", name=f"E_{pair}_{half}_{jt // 2}"
                        )
                # PV first (deps met 5 steps ago), fillers, scores LAST so
                # the PE never head-of-line blocks on an exp slot drain.
                if s >= LAG:
                    do_pv_step(s)
                for ot in TRANSW.get(s, ()):
                    trans_w(ot)
                for ct in TRANSPW.get(s, ()):
                    trans_pw(ct)
                if s == 3:
                    bias_bcast(bv_r, bv3.rearrange("p h d -> p (h d)"))
                if s in VS:
                    v_proj(VS[s])
                if s in QKS:
                    qk_tile(QKS[s])
                if s == 19:
                    bias_bcast(pb_r, pb_bc)
                for ot in LOADW.get(s, ()):
                    load_w(ot)
                for ct in LOADPW.get(s, ()):
                    load_pw(ct)
                # normalize previous pair
                if pair >= 1 and jt == 7:
                    norm_pair(pair - 1, recs[2 * (pair - 1)], recs[2 * (pair - 1) + 1])
                scores_chunk(pair, jt)

            for s in range(48, 48 + LAG):
                do_pv_step(s)
            norm_pair(5, recs[10], recs[11])
            for it in range(NT):
                proj_tile(it)

    nc.compile()
    return nc


_NC_CACHE = None


def _get_nc():
    global _NC_CACHE
    if _NC_CACHE is None:
        _NC_CACHE = build_nc()
    return _NC_CACHE


def run(inputs, trace=False, tmpdir=None):
    """Run on 8 NeuronCores; returns (out[8,32,32,768], BassKernelResults)."""
    from concourse.bass_utils import run_bass_kernel_spmd

    x = np.asarray(inputs["x"], dtype=np.float32)
    B, H, W, Cc = x.shape
    xf = np.ascontiguousarray(x.reshape(B, H * W, Cc))
    qkv_w = np.ascontiguousarray(np.asarray(inputs["qkv_w"], dtype=np.float32))
    qkv_b = np.ascontiguousarray(np.asarray(inputs["qkv_b"], dtype=np.float32))
    proj_w = np.ascontiguousarray(np.asarray(inputs["proj_w"], dtype=np.float32))
    proj_b = np.ascontiguousarray(np.asarray(inputs["proj_b"], dtype=np.float32))

    nc = _get_nc()
    in_maps = [
        {
            "x": xf[b],
            "qkv_w": qkv_w,
            "qkv_b": qkv_b,
            "proj_w": proj_w,
            "proj_b": proj_b,
        }
        for b in range(B)
    ]
    res = run_bass_kernel_spmd(nc, in_maps, list(range(B)), trace=trace, tmpdir=tmpdir)
    out = np.stack([res.results[b]["out"] for b in range(B)])
    return out.reshape(B, H, W, Cc).astype(np.float32), res


def kernel(x, qkv_w, qkv_b, proj_w, proj_b):
    out, _ = run(
        {
            "x": x,
            "qkv_w": qkv_w,
            "qkv_b": qkv_b,
            "proj_w": proj_w,
            "proj_b": proj_b,
        }
    )
    return out
